# revision 3
# baseline (speedup 1.0000x reference)
"""Grouped-expert SwiGLU kernel v2: straight-line static PE stream.

Tokens are tile-balanced across cores in MT=512 slots. Expert weights are
selected per slot by a cond-predicated DMA with a dynamic DRAM offset: the
weights stay resident in a single SBUF buffer and are reloaded only at
slots where the expert changes (host precomputes per-slot (expert, load)
pairs in `meta`). No control flow in the device program, so DMA prefetch
and PE work pipeline freely across slots.
"""

import math
import os

import ml_dtypes
import numpy as np

D = 2048
F = 512
MT = 512
KC = D // 128
FC = F // 128
NCORES = 8
E = 8

_cache = {}


def _build(nt: int, smax: int, reps: int = 1):
    import concourse.bacc as bacc
    import concourse.bass as bass
    import concourse.mybir as mybir
    from concourse.tile import TileContext

    dt = mybir.dt
    f32 = dt.float32
    bf16 = dt.bfloat16
    i32 = dt.int32
    AF = mybir.ActivationFunctionType

    nc = bacc.Bacc(
        "TRN2", target_bir_lowering=False, debug=False,
        enable_asserts=False, num_devices=NCORES,
    )

    XH = nc.dram_tensor("xh", [128, nt * KC * MT], bf16, kind="ExternalInput")
    W1T = nc.dram_tensor("w1t", [smax, 128, KC * F], bf16, kind="ExternalInput")
    W3T = nc.dram_tensor("w3t", [smax, 128, KC * F], bf16, kind="ExternalInput")
    W2T = nc.dram_tensor("w2t", [smax, 128, FC * D], bf16, kind="ExternalInput")
    META = nc.dram_tensor("meta", [1, 2 * nt], i32, kind="ExternalInput")
    OUT = nc.dram_tensor("out", [nt * MT, D], bf16, kind="ExternalOutput")

    with TileContext(nc) as tc:
        with (
            tc.tile_pool(name="wp", bufs=1) as wp,
            tc.tile_pool(name="xp", bufs=4) as xp,
            tc.tile_pool(name="hp", bufs=2) as hp,
            tc.tile_pool(name="sl", bufs=4) as slp,
            tc.tile_pool(name="op", bufs=4) as op,
            tc.tile_pool(name="mp", bufs=1) as mp,
            tc.tile_pool(name="ps", bufs=8, space="PSUM") as ps,
        ):
            msb = mp.tile([1, 2 * nt], i32, tag="meta")
            nc.sync.dma_start(out=msb[:], in_=META.ap())

            w1s = wp.tile([128, KC, F], bf16, tag="w1s")
            w3s = wp.tile([128, KC, F], bf16, tag="w3s")
            w2s = wp.tile([128, FC, D], bf16, tag="w2s")

            evs, lvs = [], []
            for m in range(nt):
                evs.append(nc.snap(nc.values_load(
                    msb[0:1, 2 * m:2 * m + 1],
                    min_val=0, max_val=smax - 1,
                    skip_runtime_bounds_check=True)))
                lvs.append(nc.snap(nc.values_load(
                    msb[0:1, 2 * m + 1:2 * m + 2],
                    min_val=0, max_val=1,
                    skip_runtime_bounds_check=True)))

            for r in range(reps):
                for m in range(nt):
                    ev, lv = evs[m], lvs[m]
                    first = (r == 0 and m == 0)
                    wargs = ({} if first
                             else {"cond": lv, "cond_hint": m == 0})
                    nc.sync.dma_start(out=w1s[:],
                                      in_=W1T.ap()[bass.ds(ev, 1)], **wargs)
                    nc.sync.dma_start(out=w3s[:],
                                      in_=W3T.ap()[bass.ds(ev, 1)], **wargs)
                    nc.sync.dma_start(out=w2s[:],
                                      in_=W2T.ap()[bass.ds(ev, 1)], **wargs)

                    xt = xp.tile([128, KC, MT], bf16, tag="xt")
                    nc.sync.dma_start(
                        out=xt[:],
                        in_=XH[:, m * KC * MT:(m + 1) * KC * MT]
                        .rearrange("p (k t) -> p k t", k=KC))

                    ht = hp.tile([128, FC, MT], bf16, tag="ht")
                    for f in range(FC):
                        x1t = ps.tile([128, MT], f32, tag="ps")
                        x3t = ps.tile([128, MT], f32, tag="ps")
                        for k in range(KC):
                            lhs1 = w1s[:, k, f * 128:(f + 1) * 128]
                            lhs3 = w3s[:, k, f * 128:(f + 1) * 128]
                            rhs = xt[:, k, :]
                            nc.tensor.matmul(x1t[:], lhs1, rhs,
                                             start=(k == 0), stop=(k == KC - 1))
                            nc.tensor.matmul(x3t[:], lhs3, rhs,
                                             start=(k == 0), stop=(k == KC - 1))
                        sil = slp.tile([128, MT], f32, tag="sil")
                        nc.scalar.activation(sil[:], x1t[:], AF.Silu)
                        nc.vector.tensor_mul(ht[:, f, :], sil[:], x3t[:])

                    for ts in range(4):
                        po = []
                        for _ in range(4):
                            pot = ps.tile([128, 512], f32, tag="ps")
                            po.append(pot)
                        for fc in range(FC):
                            lhs = ht[:, fc, ts * 128:(ts + 1) * 128]
                            for dc in range(4):
                                nc.tensor.matmul(
                                    po[dc][:], lhs,
                                    w2s[:, fc, dc * 512:(dc + 1) * 512],
                                    start=(fc == 0), stop=(fc == FC - 1))
                        osb = op.tile([128, D], bf16, tag="osb")
                        for dc in range(4):
                            nc.vector.tensor_copy(
                                osb[:, dc * 512:(dc + 1) * 512], po[dc][:])
                        nc.sync.dma_start(
                            out=OUT[m * MT + ts * 128:m * MT + (ts + 1) * 128, :],
                            in_=osb[:])

    nc.compile()
    return nc


def _get_program(nt: int, smax: int, reps: int = 1):
    key = (nt, smax, reps)
    if key not in _cache:
        _cache[key] = _build(nt, smax, reps)
    return _cache[key]


def _assign(counts):
    """Greedy: chunk the padded-tile list into per-core runs of <=NT tiles
    spanning <=2 experts when possible. Returns (nt, per-core list of
    (expert, tile_lo, n_tiles) segments)."""
    En = len(counts)
    pt = [max(1, math.ceil(c / MT)) if c > 0 else 0 for c in counts]
    total = sum(pt)
    nt = math.ceil(total / NCORES)
    for nt_try in (nt, nt + 1):
        segs = [[] for _ in range(NCORES)]
        e, used = 0, 0
        for c in range(NCORES):
            cap = nt_try
            nexp = 0
            while cap > 0 and e < En:
                if pt[e] - used == 0:
                    e += 1
                    used = 0
                    continue
                if nexp == 2:
                    break
                take = min(cap, pt[e] - used)
                segs[c].append((e, used, take))
                used += take
                cap -= take
                nexp += 1
        leftover = total - sum(s[2] for core in segs for s in core)
        if leftover == 0:
            return nt_try, segs
    # fallback: linear chunking, any number of experts per core
    flat = []
    for e in range(En):
        flat += [e] * pt[e]
    nt = math.ceil(total / NCORES)
    segs = [[] for _ in range(NCORES)]
    for c in range(NCORES):
        chunk = flat[c * nt:(c + 1) * nt]
        i = 0
        while i < len(chunk):
            e = chunk[i]
            j = i
            while j < len(chunk) and chunk[j] == e:
                j += 1
            prior = flat[:c * nt].count(e)
            segs[c].append((e, prior, j - i))
            i = j
    return nt, segs


def kernel(x, num_tokens_per_expert, w1, w2, w3):
    from concourse.bass_utils import run_bass_kernel_spmd

    x = np.asarray(x)
    counts = [int(v) for v in np.asarray(num_tokens_per_expert)]
    w1 = np.asarray(w1)
    w2 = np.asarray(w2)
    w3 = np.asarray(w3)
    T, En = x.shape[0], len(counts)
    starts = np.concatenate([[0], np.cumsum(counts)])[:En].astype(np.int64)

    nt, segs = _assign(counts)
    smax = max(2, max(len({s[0] for s in core}) for core in segs if core))
    nc = _get_program(nt, smax)

    bf = ml_dtypes.bfloat16
    # weights pre-transposed to [E, 128, KC*F] / [E, 128, FC*D] so the
    # per-slot weight DMA is contiguous per partition
    w1t_full = np.ascontiguousarray(
        w1.astype(bf).reshape(En, KC, 128, F).transpose(0, 2, 1, 3)
    ).reshape(En, 128, KC * F)
    w3t_full = np.ascontiguousarray(
        w3.astype(bf).reshape(En, KC, 128, F).transpose(0, 2, 1, 3)
    ).reshape(En, 128, KC * F)
    w2t_full = np.ascontiguousarray(
        w2.astype(bf).reshape(En, FC, 128, D).transpose(0, 2, 1, 3)
    ).reshape(En, 128, FC * D)
    xb = x.astype(bf)

    in_maps = []
    placements = []  # per core: list of (slot, src_lo, nrows)
    for c in range(NCORES):
        cs = segs[c]
        exps = []
        for (e, _, _) in cs:
            if e not in exps:
                exps.append(e)
        emap = {e: i for i, e in enumerate(exps)}
        while len(exps) < smax:
            exps.append(exps[-1] if exps else 0)

        xh = np.zeros((128, nt, KC, MT), dtype=bf)
        meta = np.zeros((1, 2 * nt), np.int32)
        place = []
        slot = 0
        prev_e = None
        for (e, tile_lo, ntk) in cs:
            src_lo = int(starts[e]) + tile_lo * MT
            src_hi = min(int(starts[e]) + counts[e], src_lo + ntk * MT)
            for tk in range(ntk):
                lo = src_lo + tk * MT
                nrow = max(0, min(MT, src_hi - lo))
                if nrow > 0:
                    blk = xb[lo:lo + nrow].reshape(nrow, KC, 128)
                    xh[:, slot, :, :nrow] = blk.transpose(2, 1, 0)
                    place.append((slot, lo, nrow))
                meta[0, 2 * slot] = emap[e]
                meta[0, 2 * slot + 1] = 1 if e != prev_e else 0
                prev_e = e
                slot += 1
        while slot < nt:
            meta[0, 2 * slot] = emap[cs[-1][0]] if cs else 0
            meta[0, 2 * slot + 1] = 0
            slot += 1
        placements.append(place)
        in_maps.append({
            "xh": np.ascontiguousarray(xh.reshape(128, nt * KC * MT)),
            "w1t": np.ascontiguousarray(w1t_full[exps]),
            "w3t": np.ascontiguousarray(w3t_full[exps]),
            "w2t": np.ascontiguousarray(w2t_full[exps]),
            "meta": meta,
        })

    res = run_bass_kernel_spmd(nc, in_maps, core_ids=list(range(NCORES)))
    kernel.last_results = res
    kernel.last_programs = (nc, in_maps)

    out = np.empty((T, D), dtype=np.float32)
    for c in range(NCORES):
        o = res.results[c]["out"]
        for (slot, src_lo, nrow) in placements[c]:
            out[src_lo:src_lo + nrow] = o[slot * MT:slot * MT + nrow].astype(
                np.float32)
    return out
